# revision 21
# baseline (speedup 1.0000x reference)
"""Trainium2 Bass kernel for nn_BDH_4406636445711 (dense transformer).

Sharding: 8 cores = data-parallel over B(2) x tensor-parallel over H(4).
Core c handles (b = c//4, h = c%4): its head's Dx/Dy slices, E rows, and a
V/4 shard of the readout. Per layer the y@E partial is AllReduced within
each b-group of 4 cores; v stays replicated inside the group. The host
stitches the 8 per-core [VS, T] logit shards into the full [B, T, V].

Key algorithmic change vs the naive reference: there is no softmax, so
  a = (q @ q^T) @ v  ==  q @ (q^T @ v)
which replaces the [T,T] scores matmul (8.6 GF/core/layer) with two
[T,K]x[K,D]-sized matmuls (1.1 GF each). M = q^T(v+pos) is computed from
td-layout tiles built by DMA-transpose (bf16 xbar path), costing zero PE
time. q/x/Dx/Dy/E/M and the readout run in bf16; the residual stream v
and all LayerNorm statistics stay f32/f32r. The per-token LN(a) rstd is
folded into the y@E psum->sbuf scale; the mean is centered with one bf16
DVE pass. The B phase is split per t-half and runs between the two E
phases so the second AllReduce of each layer is hidden under next-layer
compute. The C2 accumulation (a = M^T q^T, th0) is interleaved into the
M loop per k-tile so the C1->C2 boundary costs no PE idle.

Note: matmul start=True zeroes the full PSUM bank, so the per-kc M
chains that share banks complete (both t-halves) and are copied out
before the bank-mate chain starts (th inside kc).
"""

import os
import sys

sys.path.insert(0, "/opt/trn_rl_repo")

import ml_dtypes
import numpy as np

import concourse.bass as bass
import concourse.tile as tile
from concourse import bacc, mybir
from concourse.bass_utils import run_bass_kernel_spmd
from concourse.masks import make_identity
from concourse import library_config

F32 = mybir.dt.float32
F32R = mybir.dt.float32r
BF16 = mybir.dt.bfloat16
I32 = mybir.dt.int32
AF = mybir.ActivationFunctionType
OP = mybir.AluOpType

B, T, H, D, K, V, L = 2, 2048, 4, 256, 1024, 32000, 6
VS = V // 4          # vocab shard per core within a b-group
EPS = 1e-5
NT = T // 128        # 16 token tiles
NKT = K // 128       # 8 k' tiles
ND = D // 128        # 2 d tiles
TH = T // 2          # t-half = 1024
NS = TH // 512       # 512-wide matmul chunks per t-half
KC_ORDER = (0, 4, 1, 5, 2, 6, 3, 7)

N_LAYERS = int(os.environ.get("KRN_LAYERS", str(L)))
DO_READOUT = os.environ.get("KRN_READOUT", "1") == "1"


def build(nc):
    # ---- DRAM parameters (per core) ----
    tok_d = nc.dram_tensor("tok", [T], I32, kind="ExternalInput")
    emb_d = nc.dram_tensor("emb", [V, D], F32, kind="ExternalInput")
    posT_d = nc.dram_tensor("posT", [D, T], F32, kind="ExternalInput")
    dx_d = nc.dram_tensor("dx", [D, K], BF16, kind="ExternalInput")
    dy_d = nc.dram_tensor("dy", [D, K], BF16, kind="ExternalInput")
    e_d = nc.dram_tensor("eh", [K, D], BF16, kind="ExternalInput")
    ro_d = nc.dram_tensor("ro", [D, VS], BF16, kind="ExternalInput")
    trig_d = nc.dram_tensor("trig", [4, 2, 128, T], BF16, kind="ExternalInput")
    out_d = nc.dram_tensor("logitsT", [VS, T], BF16, kind="ExternalOutput")

    groups = [[0, 1, 2, 3], [4, 5, 6, 7]]

    with tile.TileContext(nc) as tc:
        with (
            nc.allow_low_precision(reason="bf16/f32r rounding is intentional"),
            tc.tile_pool(name="persist", bufs=1) as pp,
            tc.tile_pool(name="work", bufs=1) as wp,
            tc.tile_pool(name="ps", bufs=4, space="PSUM") as psp,
            tc.tile_pool(name="dram", bufs=1, space="DRAM") as dpool,
        ):
            _ctr = [0]

            def _nm(p):
                _ctr[0] += 1
                return f"{p}{_ctr[0]}"

            # pool helpers -- tags control slot sharing
            def xt_t():
                # [128, TH] bf16 x tiles
                return wp.tile([128, TH], BF16, tag="xt", bufs=4, name=_nm("xt_"))

            def rope_t():
                return wp.tile([128, TH], BF16, tag="rope", bufs=2, name=_nm("rp_"))

            def trig_t():
                # [128, 2, TH] bf16: cos/sin pair for one k-block
                return wp.tile([128, 2, TH], BF16, tag="trig", bufs=3,
                               name=_nm("tg_"))

            def qtd_t():
                return wp.tile([128, 8, 128], BF16, tag="qtd", bufs=3,
                               name=_nm("qtd_"))

            def f4_t():
                # [128, TH] f32 scratch (E-phase temps)
                return wp.tile([128, TH], F32, tag="f4", bufs=2, name=_nm("f4_"))

            def xr_t():
                # [128, TH] bf16 (x reload, relu result, gated y)
                return wp.tile([128, TH], BF16, tag="xr", bufs=5, name=_nm("xr_"))

            def aT_t():
                return wp.tile([128, ND, TH], BF16, tag="aT", bufs=2,
                               name=_nm("aT_"))

            def sq_t():
                return wp.tile([128, ND, TH], BF16, tag="sq", bufs=2,
                               name=_nm("sq_"))

            def rs_t():
                # [128, TH] f32 broadcast tiles (rstd)
                return wp.tile([128, TH], F32, tag="rs", bufs=2, name=_nm("rs_"))

            def nmb_t():
                # [128, TH] bf16 broadcast tiles (a-site negmean)
                return wp.tile([128, TH], BF16, tag="nmb", bufs=2, name=_nm("nm_"))

            def w8_t():
                # [128, ND, TH] f32 (uT, ye, pos pair, readout logits)
                return wp.tile([128, ND, TH], F32, tag="w8", bufs=3,
                               name=_nm("w8_"))

            def st_t(dt=F32):
                return wp.tile([1, TH], dt, tag="st", bufs=3, name=_nm("st_"))

            def ps4(shape=None, dt=F32):
                return psp.tile(shape or [128, TH], dt, tag="ps4", name=_nm("ps_"))

            # ---- constants ----
            ident_f = wp.tile([128, 128], F32, tag="idf", bufs=1)
            make_identity(nc, ident_f[:])
            ident_r = pp.tile([128, 128], F32R)
            nc.vector.tensor_copy(ident_r[:], ident_f[:])
            ones_pf = pp.tile([128, 1], F32)
            nc.vector.memset(ones_pf[:], 1.0)
            ones_p = pp.tile([128, 1], F32R)
            nc.vector.tensor_copy(ones_p[:], ones_pf[:])
            ones_pb = pp.tile([128, 1], BF16)
            nc.vector.tensor_copy(ones_pb[:], ones_pf[:])
            ones_cf = pp.tile([1, 128], F32)
            nc.vector.memset(ones_cf[:], 1.0)
            ones_c = pp.tile([1, 128], F32R)
            nc.vector.tensor_copy(ones_c[:], ones_cf[:])
            eps_p = pp.tile([128, 1], F32)
            nc.vector.memset(eps_p[:], EPS)
            eps_1 = pp.tile([1, 1], F32)
            nc.vector.memset(eps_1[:], EPS)
            zro_st = pp.tile([1, 128], BF16)
            nc.vector.memset(zro_st[:], 0.0)
            zro_mv = pp.tile([1, 512], BF16)
            nc.vector.memset(zro_mv[:], 0.0)
            nc.gpsimd.load_library(library_config.attn)

            # ---- persistent tensors ----
            vT = pp.tile([128, ND, T], F32R)       # v (dT layout), f32 bits
            vpb = pp.tile([128, ND, T], BF16)      # bf16 copy of v (dT layout)
            qT = pp.tile([128, NKT, T], BF16)      # q (kT layout)
            vp_td = pp.tile([128, NT, ND, 128], BF16)  # v+pos (td, xbar tiling)
            m_sb = pp.tile([128, NKT, D], BF16)    # M = q^T (v+pos)  [k, d]
            dx_sb = pp.tile([128, ND, K], BF16)
            nc.sync.dma_start(dx_sb[:], dx_d.ap().rearrange("(c p) k -> p c k", p=128))
            dy_sb = pp.tile([128, ND, K], BF16)
            nc.sync.dma_start(dy_sb[:], dy_d.ap().rearrange("(c p) k -> p c k", p=128))
            e_sb = pp.tile([128, NKT, D], BF16)
            nc.sync.dma_start(e_sb[:], e_d.ap().rearrange("(c p) d -> p c d", p=128))

            # ---- internal DRAM ----
            xspill = dpool.tile([NKT, 128, T], BF16, tag="xspill")
            cc_in = [dpool.tile([ND, 128, TH], BF16, tag=f"cci{i}", name=f"cci{i}")
                     for i in range(2)]
            cc_out = [dpool.tile([ND, 128, TH], BF16, tag=f"cco{i}", name=f"cco{i}")
                      for i in range(2)]

            def alloc_pms():
                # matmul start=True zeroes the full psum bank; the M chains
                # share banks across kc, so pre-zero each bank once and
                # accumulate with start=False.
                pms = [ps4([128, 4, 256]), ps4([128, 4, 256])]
                for pmt in pms:
                    for half in range(2):
                        nc.tensor.matmul(
                            pmt[:, half * 2:(half + 1) * 2, :],
                            zro_st[:], zro_mv[:],
                            start=True, stop=False, skip_group_check=True)
                return pms

            def c1_half(pms, th, with_c2=None):
                """M += q_td^T vp_td over one t-half (xbar-transposed q).

                With with_c2 = (tq0, pa0), per k-tile also emit the M
                psum->sbuf copy and the C2 chain pa0 += M[kc]^T q[kc] for
                the th0 t-half."""
                t0 = th * TH
                for j, kc in enumerate(KC_ORDER):
                    pm = pms[kc // 4][:, kc % 4, :]
                    qtd = qtd_t()
                    nc.sync.dma_start(
                        out=qtd[:], in_=qT[:, kc, t0:t0 + TH], transpose=True)
                    for n in range(8):
                        nc.tensor.matmul(
                            pm, qtd[:, n, :], vp_td[:, th * 8 + n, :, :],
                            start=False, stop=(th == 1 and n == 7),
                            skip_group_check=True)
                    if with_c2 is not None:
                        tq0, pa0 = with_c2
                        if j % 2 == 0:
                            nc.vector.tensor_copy(m_sb[:, kc, :], pm)
                        else:
                            nc.scalar.copy(m_sb[:, kc, :], pm)
                        for dc in range(ND):
                            for ns in range(NS):
                                nc.tensor.matmul(
                                    pa0[dc][:, ns * 512:(ns + 1) * 512],
                                    m_sb[:, kc, dc * 128:(dc + 1) * 128],
                                    qT[:, kc, tq0 + ns * 512:tq0 + (ns + 1) * 512],
                                    start=(kc == 0), stop=(kc == 7),
                                    skip_group_check=True)

            def colsum(mov_fn, ones):
                """s[t] = sum_d mov[d, t] over ND tiles -> [1, TH] psum AP."""
                s = ps4([1, TH])
                for dc in range(ND):
                    for ns in range(NS):
                        nc.tensor.matmul(
                            s[:1, ns * 512:(ns + 1) * 512], ones[:],
                            mov_fn(dc, ns * 512, (ns + 1) * 512),
                            start=(dc == 0), stop=(dc == ND - 1),
                            skip_group_check=True)
                return s[:1, :]

            def bcast(vec):
                """PE rank-1 broadcast of a [1, TH] f32r vector to [128, TH] psum."""
                out = ps4()
                for ns in range(NS):
                    nc.tensor.matmul(out[:, ns * 512:(ns + 1) * 512], ones_c[:],
                                     vec[:, ns * 512:(ns + 1) * 512],
                                     start=True, stop=True)
                return out

            def lnstats(s1_ps, s2_fn, n):
                """LN stats chain -> (nm_ps, rs_ps) [128, TH] psum bcasts.

                The negmean broadcast is issued early so centered-value
                consumers can start before rstd is ready."""
                negmean = st_t(F32R)
                nc.vector.tensor_scalar_mul(negmean[:], s1_ps, -1.0 / n)
                nm_ps = bcast(negmean)
                m2 = st_t()
                nc.vector.tensor_mul(m2[:], negmean[:].bitcast(F32),
                                     negmean[:].bitcast(F32))
                s2 = s2_fn()
                var = st_t()
                nc.vector.scalar_tensor_tensor(
                    out=var[:], in0=s2, scalar=1.0 / n, in1=m2[:],
                    op0=OP.mult, op1=OP.subtract)
                lnv = st_t()
                nc.scalar.activation(lnv[:], var[:], AF.Ln, bias=eps_1[:])
                rstd = st_t(F32R)
                nc.scalar.activation(rstd[:], lnv[:], AF.Exp, scale=-0.5)
                rs_ps = bcast(rstd)
                return nm_ps, rs_ps

            def lnstats_const_nm(negmean_bf, s2_fn, n):
                """lnstats with a precomputed [1, TH] bf16 negmean."""
                nm_ps = ps4()
                for ns in range(NS):
                    nc.tensor.matmul(nm_ps[:, ns * 512:(ns + 1) * 512],
                                     ones_cb[:],
                                     negmean_bf[:, ns * 512:(ns + 1) * 512],
                                     start=True, stop=True)
                m2 = st_t()
                nc.vector.tensor_mul(m2[:], negmean_bf, negmean_bf)
                s2 = s2_fn()
                var = st_t()
                nc.vector.scalar_tensor_tensor(
                    out=var[:], in0=s2, scalar=1.0 / n, in1=m2[:],
                    op0=OP.mult, op1=OP.subtract)
                lnv = st_t()
                nc.scalar.activation(lnv[:], var[:], AF.Ln, bias=eps_1[:])
                rstd = st_t(F32R)
                nc.scalar.activation(rstd[:], lnv[:], AF.Exp, scale=-0.5)
                rs_ps = bcast(rstd)
                return nm_ps, rs_ps

            # ============ embedding gather + LN -> v0 -> transpose to vT ============
            idx = pp.tile([128, NT], I32)
            nc.sync.dma_start(idx[:], tok_d.ap().rearrange("(n p) -> p n", p=128))
            for n in range(NT):
                gat = wp.tile([128, D], F32, tag="gat", bufs=2, name=_nm("g_"))
                nc.gpsimd.indirect_dma_start(
                    out=gat[:], out_offset=None, in_=emb_d.ap(),
                    in_offset=bass.IndirectOffsetOnAxis(ap=idx[:, n:n + 1], axis=0),
                )
                stats = wp.tile([128, 6], F32, tag="bst", bufs=2, name=_nm("g_"))
                nc.vector.bn_stats(out=stats[:], in_=gat[:])
                mv = wp.tile([128, 2], F32, tag="bmv", bufs=2, name=_nm("g_"))
                nc.vector.bn_aggr(out=mv[:], in_=stats[:])
                std = wp.tile([128, 1], F32, tag="bsd", bufs=2, name=_nm("g_"))
                nc.scalar.activation(std[:], mv[:, 1:2], AF.Sqrt, bias=eps_p[:])
                rstd = wp.tile([128, 1], F32, tag="brs", bufs=2, name=_nm("g_"))
                nc.vector.reciprocal(rstd[:], std[:])
                v0 = wp.tile([128, D], F32R, tag="gv0", bufs=2, name=_nm("g_"))
                nc.vector.tensor_scalar(
                    out=v0[:], in0=gat[:], scalar1=mv[:, 0:1], scalar2=rstd[:],
                    op0=OP.subtract, op1=OP.mult)
                for dc in range(ND):
                    tp = ps4([128, 128], F32R)
                    nc.tensor.transpose(out=tp[:], in_=v0[:, dc * 128:(dc + 1) * 128],
                                        identity=ident_r[:])
                    nc.vector.tensor_copy(vT[:, dc, n * 128:(n + 1) * 128], tp[:])

            posT_r = posT_d.ap().rearrange("(c p) t -> p c t", p=128)

            # negmean of w = v+pos+u is -colsum(pos)/D: v and u are LN-centered
            negmw = pp.tile([1, T], BF16)
            ones_cb = pp.tile([1, 128], BF16)
            nc.vector.tensor_copy(ones_cb[:], ones_cf[:])
            for th0_ in range(2):
                pchc = w8_t()
                nc.sync.dma_start(pchc[:], posT_r[:, :, th0_ * TH:(th0_ + 1) * TH])
                cp = ps4([1, TH])
                for dc in range(ND):
                    for ns in range(NS):
                        nc.tensor.matmul(
                            cp[:1, ns * 512:(ns + 1) * 512], ones_pf[:],
                            pchc[:, dc, ns * 512:(ns + 1) * 512],
                            start=(dc == 0), stop=(dc == ND - 1),
                            skip_group_check=True)
                nc.vector.tensor_scalar_mul(
                    negmw[:1, th0_ * TH:(th0_ + 1) * TH], cp[:1, :], -1.0 / D)

            def phaseA(th):
                """v[:, th-half] += pos; refresh vpb + vp_td for that half."""
                t0 = th * TH
                pch = w8_t()
                nc.sync.dma_start(pch[:], posT_r[:, :, t0:t0 + TH])
                for dc in range(ND):
                    nc.vector.tensor_add(
                        vT[:, dc, t0:t0 + TH],
                        vT[:, dc, t0:t0 + TH].bitcast(F32), pch[:, dc])
                for dc in range(ND):
                    nc.scalar.copy(vpb[:, dc, t0:t0 + TH],
                                   vT[:, dc, t0:t0 + TH].bitcast(F32))
                    nc.sync.dma_start(
                        out=vp_td[:, th * 8:(th + 1) * 8, dc, :],
                        in_=vpb[:, dc, t0:t0 + TH],
                        transpose=True)

            def bphase(th):
                """x[:, th-half] = relu((v+pos) @ Dx); RoPE -> q; spill x."""
                t0 = th * TH
                for i in range(4):
                    tg = trig_t()
                    nc.sync.dma_start(
                        tg[:],
                        trig_d.ap()[i].rearrange("c p t -> p c t")[:, :, t0:t0 + TH])
                    xts = {}
                    for ii in (i, i + 4):
                        xt = xt_t()
                        xts[ii] = xt
                        px = ps4()
                        for dc in range(ND):
                            for ns in range(NS):
                                nc.tensor.matmul(
                                    px[:, ns * 512:(ns + 1) * 512],
                                    dx_sb[:, dc, ii * 128:(ii + 1) * 128],
                                    vpb[:, dc, t0 + ns * 512:t0 + (ns + 1) * 512],
                                    start=(dc == 0), stop=(dc == ND - 1))
                        nc.scalar.activation(xt[:], px[:], AF.Relu)
                        nc.sync.dma_start(
                            out=xspill[ii, :, t0:t0 + TH], in_=xt[:])
                    xi, xj = xts[i], xts[i + 4]
                    m1 = rope_t()
                    nc.vector.tensor_mul(m1[:], xi[:], tg[:, 0])
                    m2 = rope_t()
                    nc.vector.tensor_mul(m2[:], xj[:], tg[:, 1])
                    nc.vector.tensor_sub(qT[:, i, t0:t0 + TH], m1[:], m2[:])
                    m3 = rope_t()
                    nc.vector.tensor_mul(m3[:], xj[:], tg[:, 0])
                    m4 = rope_t()
                    nc.vector.tensor_mul(m4[:], xi[:], tg[:, 1])
                    nc.vector.tensor_add(qT[:, i + 4, t0:t0 + TH], m3[:], m4[:])

            def dphase(th, aT, rs_s):
                """y = relu((a-mu)@Dy)*x; ye = rstd * (E^T y); AllReduce."""
                t0 = th * TH
                pyes = [ps4(), ps4()]
                for i in range(NKT):
                    py = ps4()
                    for dc in range(ND):
                        for ns in range(NS):
                            nc.tensor.matmul(
                                py[:, ns * 512:(ns + 1) * 512],
                                dy_sb[:, dc, i * 128:(i + 1) * 128],
                                aT[:, dc, ns * 512:(ns + 1) * 512],
                                start=(dc == 0), stop=(dc == ND - 1))
                    rl = xr_t()
                    nc.scalar.activation(rl[:], py[:], AF.Relu)
                    xr = xr_t()
                    nc.sync.dma_start(xr[:], xspill[i, :, t0:t0 + TH])
                    yt = xr_t()
                    nc.vector.tensor_mul(yt[:], rl[:], xr[:])
                    for dc in range(ND):
                        for ns in range(NS):
                            nc.tensor.matmul(
                                pyes[dc][:, ns * 512:(ns + 1) * 512],
                                e_sb[:, i, dc * 128:(dc + 1) * 128],
                                yt[:, ns * 512:(ns + 1) * 512],
                                start=(i == 0), stop=(i == NKT - 1),
                                skip_group_check=True)
                ye = wp.tile([128, ND, TH], BF16, tag="w8", bufs=3,
                             name=_nm("ye_"))
                for dc in range(ND):
                    nc.vector.tensor_mul(ye[:, dc], pyes[dc][:], rs_s[:])
                nc.sync.dma_start(
                    cc_in[th][:].rearrange("a p t -> p a t"), ye[:])
                nc.gpsimd.collective_compute(
                    "AllReduce", OP.add, replica_groups=groups,
                    ins=[cc_in[th][:].opt()], outs=[cc_out[th][:].opt()])

            def ephase(th):
                """u = ln(AR sum); v += u; v = ln(v)  (all dT layout, f32)."""
                t0 = th * TH
                uT = wp.tile([128, ND, TH], BF16, tag="w8", bufs=3,
                             name=_nm("uT_"))
                nc.sync.dma_start(
                    uT[:], cc_out[th][:].rearrange("a p t -> p a t"))
                squ = sq_t()
                for dc in range(ND):
                    nc.scalar.activation(squ[:, dc], uT[:, dc], AF.Square)
                s1 = colsum(lambda dc, lo, hi: uT[:, dc, lo:hi], ones_pb)
                nm_u, rs_u = lnstats(
                    s1, lambda: colsum(
                        lambda dc, lo, hi: squ[:, dc, lo:hi], ones_pb), D)
                for dc in range(ND):
                    t1 = f4_t()
                    nc.vector.tensor_add(t1[:], uT[:, dc], nm_u[:])
                    nc.vector.tensor_mul(t1[:], t1[:], rs_u[:])
                    nc.vector.tensor_add(vT[:, dc, t0:t0 + TH],
                                         vT[:, dc, t0:t0 + TH].bitcast(F32),
                                         t1[:])
                sqw = sq_t()
                for dc in range(ND):
                    nc.scalar.activation(sqw[:, dc],
                                         vT[:, dc, t0:t0 + TH].bitcast(F32),
                                         AF.Square)
                nm_w, rs_w = lnstats_const_nm(
                    negmw[:1, t0:t0 + TH],
                    lambda: colsum(
                        lambda dc, lo, hi: sqw[:, dc, lo:hi], ones_pb), D)
                for dc in range(ND):
                    nc.vector.tensor_add(vT[:, dc, t0:t0 + TH],
                                         vT[:, dc, t0:t0 + TH].bitcast(F32),
                                         nm_w[:])
                    nc.vector.tensor_mul(vT[:, dc, t0:t0 + TH],
                                         vT[:, dc, t0:t0 + TH].bitcast(F32),
                                         rs_w[:])

            # ================================ layers ================================
            # bootstrap: layer 0's B phase + first M half
            phaseA(0)
            bphase(0)
            pms = alloc_pms()
            c1_half(pms, 0)
            phaseA(1)
            bphase(1)

            for layer in range(N_LAYERS):
                # ---- C1 second half + M copies + C2(th0), interleaved ----
                aT0 = aT_t()
                sq0 = sq_t()
                pa0 = [ps4(), ps4()]
                c1_half(pms, 1, with_c2=(0, pa0))
                for dc in range(ND):
                    nc.scalar.copy(aT0[:, dc], pa0[dc][:])
                    nc.scalar.activation(sq0[:, dc], pa0[dc][:], AF.Square)

                # ---- per t-half: LN(a) stats + D (C2 for th1 computed here) ----
                for th in range(2):
                    if th == 0:
                        aT, sq = aT0, sq0
                    else:
                        aT = aT_t()
                        sq = sq_t()
                        for dc in range(ND):
                            pa = ps4()
                            for kc in range(NKT):
                                for ns in range(NS):
                                    nc.tensor.matmul(
                                        pa[:, ns * 512:(ns + 1) * 512],
                                        m_sb[:, kc, dc * 128:(dc + 1) * 128],
                                        qT[:, kc, TH + ns * 512:TH + (ns + 1) * 512],
                                        start=(kc == 0), stop=(kc == NKT - 1))
                            nc.scalar.copy(aT[:, dc], pa[:])
                            nc.scalar.activation(sq[:, dc], pa[:], AF.Square)
                    s1 = colsum(lambda dc, lo, hi: aT[:, dc, lo:hi], ones_pb)
                    nm_ps, rs_ps = lnstats(
                        s1, lambda: colsum(
                            lambda dc, lo, hi: sq[:, dc, lo:hi], ones_pb), D)
                    nm_s = nmb_t()
                    nc.scalar.copy(nm_s[:], nm_ps[:])
                    rs_s = rs_t()
                    nc.scalar.copy(rs_s[:], rs_ps[:])
                    for dc in range(ND):
                        nc.vector.tensor_add(aT[:, dc], aT[:, dc], nm_s[:])
                    dphase(th, aT, rs_s)

                # ---- E phases, interleaved with next layer's B front ----
                ephase(0)
                if layer < N_LAYERS - 1:
                    phaseA(0)
                    bphase(0)
                    pms = alloc_pms()
                    c1_half(pms, 0)
                else:
                    for dc in range(ND):
                        nc.scalar.copy(vpb[:, dc, 0:TH],
                                       vT[:, dc, 0:TH].bitcast(F32))
                ephase(1)
                if layer < N_LAYERS - 1:
                    phaseA(1)
                    bphase(1)
                else:
                    for dc in range(ND):
                        nc.scalar.copy(vpb[:, dc, TH:T],
                                       vT[:, dc, TH:T].bitcast(F32))

            # ============= readout: logitsT = (v @ readout)^T, V-sharded =============
            if DO_READOUT:
                ro_r = ro_d.ap().rearrange("(c p) v -> p c v", p=128)
                nvb = (VS + 127) // 128
                for vb in range(nvb):
                    m = min(128, VS - vb * 128)
                    ro_sb = wp.tile([128, ND, 128], BF16, tag="ro",
                                    bufs=3, name=_nm("ro_"))
                    nc.sync.dma_start(ro_sb[:, :, :m],
                                      ro_r[:, :, vb * 128:vb * 128 + m])
                    for th in range(2):
                        pl = ps4()
                        for dc in range(ND):
                            for ns in range(NS):
                                nc.tensor.matmul(
                                    pl[:m, ns * 512:(ns + 1) * 512],
                                    ro_sb[:, dc, :m],
                                    vpb[:, dc, th * TH + ns * 512:
                                        th * TH + (ns + 1) * 512],
                                    start=(dc == 0), stop=(dc == ND - 1))
                        lo = xr_t()
                        if (vb + th) % 2 == 0:
                            nc.scalar.copy(lo[:m], pl[:m])
                        else:
                            nc.vector.tensor_copy(lo[:m], pl[:m])
                        eng = nc.sync if th == 0 else nc.scalar
                        eng.dma_start(
                            out_d.ap()[vb * 128:vb * 128 + m,
                                       th * TH:(th + 1) * TH],
                            lo[:m])

    nc.compile()
    return nc


_NC_CACHE = None


def _get_nc():
    global _NC_CACHE
    if _NC_CACHE is None:
        nc = bacc.Bacc("TRN2", target_bir_lowering=False, debug=False, num_devices=8)
        _NC_CACHE = build(nc)
    return _NC_CACHE


def _rope_tables():
    # match the jax reference: float32 angle computation
    inv_freq = (1.0 / (10000.0 ** (np.arange(0, K, 2, dtype=np.float32)
                                   / np.float32(K)))).astype(np.float32)
    t = np.arange(T, dtype=np.float32)
    freqs = (t[:, None] * inv_freq[None, :]).astype(np.float32)  # [T, K/2]
    cos = np.cos(freqs).astype(ml_dtypes.bfloat16)
    sin = np.sin(freqs).astype(ml_dtypes.bfloat16)
    cosT = np.ascontiguousarray(cos.T).reshape(4, 128, T)
    sinT = np.ascontiguousarray(sin.T).reshape(4, 128, T)
    trig = np.stack([cosT, sinT], axis=1)  # [4, 2, 128, T]
    return np.ascontiguousarray(trig)


def kernel(input_, emb, pos, Dx, Dy, E, readout):
    input_ = np.asarray(input_)
    emb = np.ascontiguousarray(np.asarray(emb, dtype=np.float32))
    pos = np.asarray(pos, dtype=np.float32)
    Dx = np.asarray(Dx, dtype=np.float32)
    Dy = np.asarray(Dy, dtype=np.float32)
    E = np.asarray(E, dtype=np.float32)
    readout = np.asarray(readout, dtype=np.float32)

    nc = _get_nc()
    trig = _rope_tables()
    posT = np.ascontiguousarray(pos.T)

    in_maps = []
    for c in range(8):
        b, h = divmod(c, 4)
        in_maps.append({
            "tok": np.ascontiguousarray(input_[b].astype(np.int32)),
            "emb": emb,
            "posT": posT,
            "dx": np.ascontiguousarray(Dx[h]).astype(ml_dtypes.bfloat16),
            "dy": np.ascontiguousarray(Dy[h]).astype(ml_dtypes.bfloat16),
            "eh": np.ascontiguousarray(E[h * K:(h + 1) * K]).astype(
                ml_dtypes.bfloat16),
            "ro": np.ascontiguousarray(readout[:, h * VS:(h + 1) * VS]).astype(
                ml_dtypes.bfloat16),
            "trig": trig,
        })
    trace = os.environ.get("KRN_TRACE", "0") == "1"
    res = run_bass_kernel_spmd(nc, in_maps, list(range(8)), trace=trace)
    out = np.empty((B, T, V), dtype=np.float32)
    for c in range(8):
        b, h = divmod(c, 4)
        out[b, :, h * VS:(h + 1) * VS] = np.asarray(
            res.results[c]["logitsT"], dtype=np.float32).T
    kernel._last_results = res
    return out
